# revision 1
# baseline (speedup 1.0000x reference)
"""Self-contained Trainium2 Bass kernel for nn_DecoderLayer_30855045055049.

Sharding: 2 DP groups over batch (cores 0-3 -> b=0, cores 4-7 -> b=1), 4-way
TP within each group. Mamba d_inner-sharded (512 ch/core; selective scan via
GPSIMD tensor_tensor_scan per (ch-tile, state)); xproj partial -> AllReduce;
out_proj partial stored [t, dm] -> ReduceScatter over tokens (256 tok/core).
Attention: k/v head-sharded + AllGather, then token-parallel over own 256
tokens (softmax without max-subtraction; bf16 compute; fused av+denominator
via a ones-column appended to v). FFN token-parallel with exact-integer
int8-activation x ternary-weight bf16 matmuls. Final rmsnorm token-parallel;
host assembles the token shards.
"""
import numpy as np
import ml_dtypes

B, TGT, SRC = 2, 1024, 1024
D_MODEL, D_INNER, D_STATE, D_CONV, DT_RANK, D_FF, N_HEADS = 1024, 2048, 16, 4, 64, 4096, 16
EPS = 1e-6
N_CORES, N_TP = 8, 4
CH = D_INNER // N_TP          # 512 channels/core
TOK = TGT // N_TP             # 256 tokens/core
HD = D_MODEL // N_HEADS       # 64
RG = [[0, 1, 2, 3], [4, 5, 6, 7]]

_CACHE = {}


def _bf16(x):
    return np.asarray(x, np.float32).astype(ml_dtypes.bfloat16)


def _ns(nh):
    return slice(nh * 512, (nh + 1) * 512)


def _host_prep(inputs):
    f = lambda k: np.ascontiguousarray(np.asarray(inputs[k], np.float32))
    x = f('x'); enc = f('encoder_out')
    mask = np.asarray(inputs['encoder_mask'])
    in_w = f('mamba_in_w'); conv_w = f('mamba_conv_w'); conv_b = f('mamba_conv_b')
    xproj_w = f('mamba_xproj_w'); dt_w = f('mamba_dt_w'); dt_b = f('mamba_dt_b')
    A = -np.exp(f('mamba_A_log'))
    Dp = f('mamba_D'); out_w = f('mamba_out_w')
    q_w = f('q_w'); q_b = f('q_b'); k_w = f('k_w'); k_b = f('k_b')
    v_w = f('v_w'); v_b = f('v_b'); o_w = f('o_w'); o_b = f('o_b')
    w1 = f('ffn_w1'); b1 = f('ffn_b1'); w2 = f('ffn_w2'); b2 = f('ffn_b2')
    nw1 = f('norm1_w'); nw2 = f('norm2_w'); nw3 = f('norm3_w')

    def wquant(w):
        s = max(float(np.mean(np.abs(w))), 1e-5)
        return np.clip(np.round(w / s), -1.0, 1.0), np.float32(s)

    w1q, s_w1 = wquant(w1)
    w2q, s_w2 = wquant(w2)
    mask_bias = np.where(mask, 0.0, -1e9).astype(np.float32)

    flags = dict(
        has_b1=bool(np.any(b1 != 0)), has_b2=bool(np.any(b2 != 0)),
        has_nw1=bool(np.any(nw1 != 1)), has_nw2=bool(np.any(nw2 != 1)),
        has_nw3=bool(np.any(nw3 != 1)),
        s_w1=float(s_w1), s_w2=float(s_w2),
    )

    ident = np.eye(128, dtype=np.float32)
    hscale = 1.0 / np.sqrt(HD)

    in_maps = []
    for c in range(N_CORES):
        b, r = c // N_TP, c % N_TP
        chs = slice(r * CH, (r + 1) * CH)
        toks = slice(r * TOK, (r + 1) * TOK)
        hsl = slice(r * (N_HEADS // N_TP) * HD, (r + 1) * (N_HEADS // N_TP) * HD)

        convdiag = np.zeros((16, 128, 128), np.float32)
        Ddiag = np.zeros((4, 128, 128), np.float32)
        for i in range(4):
            cw = conv_w[r * CH + i * 128: r * CH + (i + 1) * 128, 0, :]
            for k in range(D_CONV):
                np.fill_diagonal(convdiag[i * 4 + k], cw[:, k])
            np.fill_diagonal(Ddiag[i], Dp[chs][i * 128:(i + 1) * 128])
        A_cols = np.empty((128, 64), np.float32)
        for i in range(4):
            A_cols[:, i * 16:(i + 1) * 16] = A[chs][i * 128:(i + 1) * 128, :]

        m = dict(
            xT=x[b].T, x_tok=x[b, toks],
            encT_bf=_bf16(enc[b].T),
            inw_uT=in_w[chs, :].T,
            inw_resT=in_w[D_INNER:, :][chs, :].T,
            convdiag=convdiag, Ddiag=Ddiag,
            cvb=conv_b[chs].reshape(4, 128).T,
            dtb=dt_b[chs].reshape(4, 128).T,
            A_cols=A_cols,
            xprojT=xproj_w[:, chs].T,
            dtwT=dt_w[chs, :].T,
            outwT=out_w[:, chs].T,
            qwT=_bf16(q_w.T * hscale), qb=_bf16((q_b * hscale).reshape(1, -1)),
            kwT=_bf16(k_w[hsl, :].T), kb=_bf16(k_b[hsl].reshape(1, -1)),
            vwT=_bf16(v_w[hsl, :].T), vb=_bf16(v_b[hsl].reshape(1, -1)),
            owT=_bf16(o_w.T), ob=_bf16(o_b.reshape(1, -1)),
            maskb=mask_bias[b].reshape(8, 128).T,
            w1qT=_bf16(w1q.T), w2qT=_bf16(w2q.T),
            b1row=b1.reshape(1, -1), b2row=b2.reshape(1, -1),
            nw1row=nw1.reshape(1, -1), nw2row=nw2.reshape(1, -1),
            nw3row=nw3.reshape(1, -1),
            ident=ident, ident_r=ident,
            ones_f=np.ones((1, 512), np.float32),
            ones_b=_bf16(np.ones((1, 512))),
        )
        in_maps.append({k: np.ascontiguousarray(v) for k, v in m.items()})
    return in_maps, flags


def _build(flags, sim_funcs=False):
    import concourse.bacc as bacc
    import concourse.tile as tile
    from concourse import mybir

    dt = mybir.dt
    f32, bff, f32r = dt.float32, dt.bfloat16, dt.float32r

    nc = bacc.Bacc("TRN2", target_bir_lowering=False, debug=False,
                   num_devices=N_CORES)

    def din(name, shape, d=f32):
        return nc.dram_tensor(name, shape, d, kind="ExternalInput").ap()

    D = dict(
        xT=din("xT", [1024, 1024], f32r), x_tok=din("x_tok", [256, 1024]),
        encT_bf=din("encT_bf", [1024, 1024], bff),
        inw_uT=din("inw_uT", [1024, 512], f32r),
        inw_resT=din("inw_resT", [1024, 512], f32r),
        convdiag=din("convdiag", [16, 128, 128], f32r),
        Ddiag=din("Ddiag", [4, 128, 128], f32r),
        cvb=din("cvb", [128, 4]), dtb=din("dtb", [128, 4]),
        A_cols=din("A_cols", [128, 64]),
        xprojT=din("xprojT", [512, 96], f32r), dtwT=din("dtwT", [64, 512], f32r),
        outwT=din("outwT", [512, 1024], f32r),
        qwT=din("qwT", [1024, 1024], bff), qb=din("qb", [1, 1024], bff),
        kwT=din("kwT", [1024, 256], bff), kb=din("kb", [1, 256], bff),
        vwT=din("vwT", [1024, 256], bff), vb=din("vb", [1, 256], bff),
        owT=din("owT", [1024, 1024], bff), ob=din("ob", [1, 1024], bff),
        maskb=din("maskb", [128, 8]),
        w1qT=din("w1qT", [1024, 4096], bff), w2qT=din("w2qT", [4096, 1024], bff),
        b1row=din("b1row", [1, 4096]), b2row=din("b2row", [1, 1024]),
        nw1row=din("nw1row", [1, 1024]), nw2row=din("nw2row", [1, 1024]),
        nw3row=din("nw3row", [1, 1024]),
        ident=din("ident", [128, 128]), ident_r=din("ident_r", [128, 128], f32r),
        ones_f=din("ones_f", [1, 512]),
        ones_b=din("ones_b", [1, 512], bff),
        out_tok=nc.dram_tensor("out_tok", [256, 1024], f32,
                               kind="ExternalOutput").ap(),
    )

    with tile.TileContext(nc) as tc:
        _emit(nc, tc, mybir, D, flags, sim_funcs)
    nc.compile()
    return nc


def _emit(nc, tc, mybir, D, flags, sim_funcs):
    from contextlib import ExitStack
    dt = mybir.dt
    f32, f32r, bff, i32 = dt.float32, dt.float32r, dt.bfloat16, dt.int32
    AF = mybir.ActivationFunctionType
    OP = mybir.AluOpType
    r32 = lambda ap: ap.bitcast(f32r)
    mm = nc.tensor.matmul

    es = ExitStack()
    const = es.enter_context(tc.tile_pool(name="const", bufs=1))
    persist = es.enter_context(tc.tile_pool(name="persist", bufs=1))
    dram = es.enter_context(tc.tile_pool(name="dram", bufs=1, space="DRAM"))
    mp_cm = tc.tile_pool(name="mamba_p", bufs=1)
    mp = mp_cm.__enter__()
    app = None

    def ctile(shape, d, name):
        t = const.tile(shape, d, tag=name, name=name)
        return t

    def ptile(shape, d, name):
        return persist.tile(shape, d, tag=name, name=name)

    def mtile(shape, d, name):
        return mp.tile(shape, d, tag=name, name=name)

    def atile(shape, d, name):
        return app.tile(shape, d, tag=name, name=name)
    app_cm = None

    # constants
    ident = ctile([128, 128], f32, "ident"); nc.sync.dma_start(ident[:], D['ident'][:])
    ident_r = ctile([128, 128], f32r, "ident_r")
    nc.sync.dma_start(ident_r[:], D['ident_r'][:])
    ones_f = ctile([1, 512], f32, "ones_f"); nc.sync.dma_start(ones_f[:], D['ones_f'][:])
    ones_b = ctile([1, 512], bff, "ones_b"); nc.sync.dma_start(ones_b[:], D['ones_b'][:])
    acol = ctile([128, 64], f32, "acol"); nc.sync.dma_start(acol[:], D['A_cols'][:])
    dtbt = ctile([128, 4], f32, "dtbt"); nc.sync.dma_start(dtbt[:], D['dtb'][:])
    cvbt = ctile([128, 4], f32, "cvbt"); nc.sync.dma_start(cvbt[:], D['cvb'][:])
    maskb = ctile([128, 8], f32, "maskb"); nc.sync.dma_start(maskb[:], D['maskb'][:])

    def silu(dst, src, pool, bias=None):
        if sim_funcs:
            shifted = pool.tile([dst.shape[0], dst.shape[1]], f32, tag="silu_t",
                                name="silu_t", bufs=1)
            if bias is None:
                nc.vector.tensor_copy(shifted[:], src)
            else:
                nc.vector.tensor_scalar(shifted[:], src, bias, None, OP.add)
            sg = pool.tile([dst.shape[0], dst.shape[1]], f32, tag="silu_s",
                           name="silu_s", bufs=1)
            nc.scalar.activation(sg[:], shifted[:], AF.Sigmoid)
            nc.vector.tensor_tensor(dst, sg[:], shifted[:], OP.mult)
        else:
            nc.scalar.activation(dst, src, AF.Silu,
                                 bias=0.0 if bias is None else bias)

    # DRAM bounce buffers
    proj_in = dram.tile([96, 1024], f32, tag="proj_in", name="proj_in")
    proj_out = dram.tile([96, 1024], f32, tag="proj_out", name="proj_out")
    mpart = dram.tile([1024, 1024], f32, tag="mpart", name="mpart")
    mout = dram.tile([256, 1024], f32, tag="mout", name="mout")
    ksh = dram.tile([256, 1024], bff, tag="ksh", name="ksh")
    kag = dram.tile([1024, 1024], bff, tag="kag", name="kag")
    vsh = dram.tile([1024, 256], bff, tag="vsh", name="vsh")
    vag = dram.tile([4096, 256], bff, tag="vag", name="vag")

    # ============== Phase KV (first, so AllGathers launch early) ==============
    with tc.tile_pool(name="enc", bufs=1) as encp, \
         tc.tile_pool(name="kvw", bufs=1) as kvwp, \
         tc.tile_pool(name="kvps", bufs=2, space="PSUM") as kvps, \
         tc.tile_pool(name="kvsb", bufs=3) as kvsb:
        enct = [encp.tile([128, 1024], bff, tag=f"enc{kd}", name=f"enc{kd}")
                for kd in range(8)]
        kwt = [kvwp.tile([128, 256], bff, tag=f"kw{kd}", name=f"kw{kd}")
               for kd in range(8)]
        vwt = [kvwp.tile([128, 256], bff, tag=f"vw{kd}", name=f"vw{kd}")
               for kd in range(8)]
        kbt = kvwp.tile([1, 256], bff, tag="kbt", name="kbt")
        vbt = kvwp.tile([1, 256], bff, tag="vbt", name="vbt")
        nc.sync.dma_start(kbt[:], D['kb'][:])
        nc.sync.dma_start(vbt[:], D['vb'][:])
        for kd in range(8):
            nc.sync.dma_start(enct[kd][:], D['encT_bf'][kd * 128:(kd + 1) * 128, :])
            nc.sync.dma_start(kwt[kd][:], D['kwT'][kd * 128:(kd + 1) * 128, :])
            nc.sync.dma_start(vwt[kd][:], D['vwT'][kd * 128:(kd + 1) * 128, :])
        for mt in range(2):
            for nh in range(2):
                ps = kvps.tile([128, 512], f32, tag="kv", name="kvp")
                for kd in range(8):
                    mm(ps[:], kwt[kd][:, mt * 128:(mt + 1) * 128],
                       enct[kd][:, _ns(nh)], start=(kd == 0), stop=False)
                mm(ps[:], kbt[0:1, mt * 128:(mt + 1) * 128], ones_b[0:1, 0:512],
                   start=False, stop=True)
                sb = kvsb.tile([128, 512], bff, tag="kvsb", name="kvsbt")
                nc.scalar.activation(sb[:], ps[:], AF.Copy)
                nc.sync.dma_start(
                    ksh[mt * 128:(mt + 1) * 128, _ns(nh)], sb[:])
        for st in range(8):
            ps = kvps.tile([128, 256], f32, tag="kv", name="kvp2")
            for kd in range(8):
                mm(ps[:], enct[kd][:, st * 128:(st + 1) * 128], vwt[kd][:],
                   start=(kd == 0), stop=False)
            mm(ps[:], ones_b[0:1, 0:128], vbt[:], start=False, stop=True)
            sb = kvsb.tile([128, 256], bff, tag="kvsb", name="kvsbt2")
            nc.scalar.activation(sb[:], ps[:], AF.Copy)
            nc.sync.dma_start(vsh[st * 128:(st + 1) * 128, :], sb[:])
        nc.gpsimd.collective_compute("AllGather", OP.bypass, replica_groups=RG,
                                     ins=[ksh.opt()], outs=[kag.opt()])
        nc.gpsimd.collective_compute("AllGather", OP.bypass, replica_groups=RG,
                                     ins=[vsh.opt()], outs=[vag.opt()])

    # ============== Phase M1: in_proj, conv+silu, xproj + AllReduce ==========
    u_act = [mp.tile([128, 1024], f32r, tag=f"uact{i}", name=f"uact{i}")
             for i in range(4)]
    res_s = [mtile([128, 1024], f32, f"res{i}") for i in range(4)]
    proj_f = mtile([64, 1024], f32, "proj_f")
    proj_r = mtile([64, 1024], f32r, "proj_r")

    with tc.tile_pool(name="xt", bufs=1) as xtp, \
         tc.tile_pool(name="inw", bufs=18) as inwp, \
         tc.tile_pool(name="m1ps", bufs=2, space="PSUM") as m1ps, \
         tc.tile_pool(name="upad", bufs=1) as upadp, \
         tc.tile_pool(name="cdg", bufs=2) as cdgp, \
         tc.tile_pool(name="silup", bufs=2) as silup:
        xt = [xtp.tile([128, 1024], f32r, tag=f"xt{kd}", name=f"xt{kd}")
              for kd in range(8)]
        for kd in range(8):
            nc.sync.dma_start(xt[kd][:], D['xT'][kd * 128:(kd + 1) * 128, :])
        upads = [upadp.tile([128, 1028], f32r, tag=f"upad{i}", name=f"upad{i}")
                 for i in range(4)]
        for i in range(4):
            nc.vector.memset(upads[i][:, 0:3].bitcast(f32), 0.0)
        for i in range(4):
            iwu = [inwp.tile([128, 128], f32r, tag="iw", name=f"iwu{i}_{kd}")
                   for kd in range(8)]
            for kd in range(8):
                nc.sync.dma_start(iwu[kd][:],
                                  D['inw_uT'][kd * 128:(kd + 1) * 128,
                                              i * 128:(i + 1) * 128])
            for nh in range(2):
                ps = m1ps.tile([128, 512], f32, tag="m1", name="m1u")
                for kd in range(8):
                    mm(ps[:], iwu[kd][:], xt[kd][:, _ns(nh)],
                       start=(kd == 0), stop=(kd == 7))
                nc.scalar.activation(
                    upads[i][:, 3 + nh * 512: 3 + nh * 512 + 512], ps[:], AF.Copy)
        for i in range(4):
            iwr = [inwp.tile([128, 128], f32r, tag="iw", name=f"iwr{i}_{kd}")
                   for kd in range(8)]
            for kd in range(8):
                nc.sync.dma_start(iwr[kd][:],
                                  D['inw_resT'][kd * 128:(kd + 1) * 128,
                                                i * 128:(i + 1) * 128])
            for nh in range(2):
                ps = m1ps.tile([128, 512], f32, tag="m1", name="m1r")
                for kd in range(8):
                    mm(ps[:], iwr[kd][:], xt[kd][:, _ns(nh)],
                       start=(kd == 0), stop=(kd == 7))
                nc.scalar.activation(res_s[i][:, _ns(nh)], ps[:], AF.Copy)
        for i in range(4):
            cdg = [cdgp.tile([128, 128], f32r, tag="cdg", name=f"cdg{i}_{k}",
                             bufs=5) for k in range(4)]
            for k in range(4):
                nc.sync.dma_start(cdg[k][:], D['convdiag'][i * 4 + k, :, :])
            for nh in range(2):
                ps = m1ps.tile([128, 512], f32, tag="m1", name="m1c")
                for k in range(4):
                    mm(ps[:], cdg[k][:],
                       upads[i][:, k + nh * 512: k + nh * 512 + 512],
                       start=(k == 0), stop=(k == 3))
                silu(u_act[i][:, _ns(nh)], ps[:], silup, bias=cvbt[:, i:i + 1])
        xpw = [cdgp.tile([128, 96], f32r, tag="xpw", name=f"xpw{i}", bufs=4)
               for i in range(4)]
        for i in range(4):
            nc.sync.dma_start(xpw[i][:], D['xprojT'][i * 128:(i + 1) * 128, :])
        for nh in range(2):
            ps = m1ps.tile([96, 512], f32, tag="m1", name="m1x")
            for i in range(4):
                mm(ps[:], xpw[i][:], u_act[i][:, _ns(nh)],
                   start=(i == 0), stop=(i == 3))
            sb = cdgp.tile([96, 512], f32, tag="xpsb", name="xpsb")
            nc.vector.tensor_copy(sb[:], ps[:])
            nc.sync.dma_start(proj_in[:, _ns(nh)], sb[:])
        nc.gpsimd.collective_compute("AllReduce", OP.add, replica_groups=RG,
                                     ins=[proj_in.opt()], outs=[proj_out.opt()])
        nc.sync.dma_start(proj_f[:], proj_out[0:64, :])
        nc.scalar.activation(proj_r[:], proj_f[:], AF.Copy)

    # ============== Phase M2: delta, du ==============
    delta = [mtile([128, 1024], f32, f"delta{i}") for i in range(4)]
    du = [mtile([128, 1024], f32, f"du{i}") for i in range(4)]
    with tc.tile_pool(name="m2ps", bufs=2, space="PSUM") as m2ps, \
         tc.tile_pool(name="m2sb", bufs=2) as m2sb:
        dtw = m2sb.tile([64, 512], f32r, tag="dtw", name="dtw", bufs=1)
        nc.sync.dma_start(dtw[:], D['dtwT'][:])
        for i in range(4):
            for nh in range(2):
                ps = m2ps.tile([128, 512], f32, tag="m2", name="m2d")
                mm(ps[:], dtw[:, i * 128:(i + 1) * 128],
                   proj_r[:, _ns(nh)], start=True, stop=True)
                tmp = m2sb.tile([128, 512], f32, tag="spe", name="spe")
                nc.scalar.activation(tmp[:], ps[:], AF.Exp, bias=dtbt[:, i:i + 1])
                nc.scalar.activation(delta[i][:, _ns(nh)], tmp[:], AF.Ln, bias=1.0)
            nc.vector.tensor_tensor(du[i][:], delta[i][:], u_act[i][:], OP.mult)

    # ============== Phase SCAN (s-outer; B/C via DMA broadcast) ==============
    y_gated = [mp.tile([128, 1024], f32r, tag=f"yg{i}", name=f"yg{i}")
               for i in range(4)]
    with tc.tile_pool(name="bcp", bufs=3) as bcp, \
         tc.tile_pool(name="yps", bufs=1, space="PSUM") as yps, \
         tc.tile_pool(name="ssb", bufs=2) as ssb, \
         tc.tile_pool(name="ddg", bufs=4) as ddgp:
        ys = [yps.tile([128, 1024], f32, tag=f"y{i}", name=f"y{i}")
              for i in range(4)]
        for i in range(4):
            dscale = ddgp.tile([128, 128], f32r, tag="ddg", name=f"ddg{i}")
            nc.sync.dma_start(dscale[:], D['Ddiag'][i, :, :])
            for nh in range(2):
                mm(ys[i][:, _ns(nh)], dscale[:], u_act[i][:, _ns(nh)],
                   start=True, stop=False)
        for s in range(16):
            Bt = bcp.tile([128, 1024], f32, tag="Bt", name="Bt")
            nc.sync.dma_start(Bt[:],
                              proj_out[64 + s:65 + s, :].to_broadcast((128, 1024)))
            Ct = bcp.tile([128, 1024], f32, tag="Ct", name="Ct")
            nc.sync.dma_start(Ct[:],
                              proj_out[80 + s:81 + s, :].to_broadcast((128, 1024)))
            for i in range(4):
                dA = ssb.tile([128, 1024], f32, tag="dA", name="dA")
                nc.scalar.activation(dA[:], delta[i][:], AF.Exp,
                                     scale=acol[:, i * 16 + s: i * 16 + s + 1])
                dBu = ssb.tile([128, 1024], f32, tag="dBu", name="dBu")
                nc.gpsimd.tensor_tensor(dBu[:], du[i][:], Bt[:], OP.mult)
                h = ssb.tile([128, 1024], f32, tag="h", name="h")
                nc.vector.tensor_tensor_scan(h[:], dA[:], dBu[:], 0.0,
                                             OP.mult, OP.add)
                hc = ssb.tile([128, 1024], f32r, tag="hc", name="hc")
                eng = nc.gpsimd if (s % 8 == 7) else nc.vector
                eng.tensor_tensor(hc[:], h[:], Ct[:], OP.mult)
                for nh in range(2):
                    mm(ys[i][:, _ns(nh)], ident_r[:], hc[:, _ns(nh)],
                       start=False, stop=(s == 15))
        for i in range(4):
            rs = ssb.tile([128, 1024], f32, tag="rsilu", name="rsilu", bufs=1)
            silu(rs[:], res_s[i][:], ssb)
            nc.vector.tensor_tensor(y_gated[i][:], ys[i][:], rs[:], OP.mult)

    # ============== Phase OUT_PROJ + ReduceScatter ==============
    with tc.tile_pool(name="ops", bufs=2, space="PSUM") as ops, \
         tc.tile_pool(name="osb", bufs=3) as osb, \
         tc.tile_pool(name="oww", bufs=1) as owwp:
        oww = [owwp.tile([128, 1024], f32r, tag=f"oww{i}", name=f"oww{i}")
               for i in range(4)]
        for i in range(4):
            nc.sync.dma_start(oww[i][:], D['outwT'][i * 128:(i + 1) * 128, :])
        for mt in range(8):
            for nh in range(2):
                ps = ops.tile([128, 512], f32, tag="op", name="opps")
                for i in range(4):
                    mm(ps[:], y_gated[i][:, mt * 128:(mt + 1) * 128],
                       oww[i][:, _ns(nh)], start=(i == 0), stop=(i == 3))
                sb = osb.tile([128, 512], f32, tag="osb", name="osbt")
                nc.vector.tensor_copy(sb[:], ps[:])
                nc.sync.dma_start(mpart[mt * 128:(mt + 1) * 128, _ns(nh)], sb[:])
        nc.gpsimd.collective_compute("ReduceScatter", OP.add, replica_groups=RG,
                                     ins=[mpart.opt()], outs=[mout.opt()])
    mp_cm.__exit__(None, None, None)

    # ============== rmsnorm helper ==============
    def rmsnorm(dst, r_ap, pool, nwrow_d, has_nw):
        sq = pool.tile([128, 1024], f32, tag="nsq", name="nsq")
        ssq = pool.tile([128, 1], f32, tag="nssq", name="nssq")
        nc.scalar.activation(sq[:], r_ap, AF.Square, accum_out=ssq[:])
        m = pool.tile([128, 1], f32, tag="nm", name="nm")
        nc.vector.tensor_scalar(m[:], ssq[:], 1.0 / 1024.0, EPS, OP.mult, OP.add)
        sr = pool.tile([128, 1], f32, tag="nsr", name="nsr")
        nc.scalar.activation(sr[:], m[:], AF.Sqrt)
        rinv = pool.tile([128, 1], f32, tag="nrinv", name="nrinv")
        nc.vector.reciprocal(rinv[:], sr[:])
        nc.vector.tensor_scalar(dst, r_ap, rinv[:, 0:1], None, OP.mult)
        if has_nw:
            nwr = pool.tile([1, 1024], f32, tag="nwr", name="nwr")
            nc.sync.dma_start(nwr[:], nwrow_d[:])
            nwb = pool.tile([128, 1024], f32, tag="nwb", name="nwb")
            nps = pool.tile([128, 1024], f32, tag="nwps", name="nwps",
                            space="PSUM")
            for nh in range(2):
                mm(nps[:, _ns(nh)], ones_f[0:1, 0:128],
                   nwr[0:1, _ns(nh)], start=True, stop=True)
            nc.vector.tensor_copy(nwb[:], nps[:])
            nc.vector.tensor_tensor(dst, dst, nwb[:], OP.mult)

    # ============== Phase H1 ==============
    app_cm = tc.tile_pool(name="attn_p", bufs=1)
    app = app_cm.__enter__()
    h1 = [atile([128, 1024], f32, f"h1_{t}") for t in range(2)]
    h1T = [atile([128, 256], bff, f"h1T{d}") for d in range(8)]
    with tc.tile_pool(name="h1p", bufs=2) as h1p, \
         tc.tile_pool(name="h1ps", bufs=2, space="PSUM") as h1ps:
        for t in range(2):
            mo = h1p.tile([128, 1024], f32, tag="mo", name="mo")
            nc.sync.dma_start(mo[:], mout[t * 128:(t + 1) * 128, :])
            xtk = h1p.tile([128, 1024], f32, tag="xtk", name="xtk")
            nc.sync.dma_start(xtk[:], D['x_tok'][t * 128:(t + 1) * 128, :])
            r1 = h1p.tile([128, 1024], f32, tag="r1", name="r1")
            nc.vector.tensor_tensor(r1[:], xtk[:], mo[:], OP.add)
            rmsnorm(h1[t][:], r1[:], h1p, D['nw1row'], flags['has_nw1'])
        for d in range(8):
            ps = h1ps.tile([128, 256], f32, tag="tr", name="trp")
            for t in range(2):
                nc.tensor.transpose(ps[:, t * 128:(t + 1) * 128],
                                    h1[t][:, d * 128:(d + 1) * 128], ident[:])
            nc.scalar.activation(h1T[d][:], ps[:], AF.Copy)

    # ============== Phase ATTN ==============
    h2 = [ptile([128, 1024], f32, f"h2_{t}") for t in range(2)]
    with tc.tile_pool(name="kvf", bufs=1) as kvf, \
         tc.tile_pool(name="attw", bufs=1) as attwp, \
         tc.tile_pool(name="asb", bufs=3) as asb, \
         tc.tile_pool(name="h2p", bufs=2) as h2p:
        kf = [kvf.tile([128, 1024], bff, tag=f"kf{kt}", name=f"kf{kt}")
              for kt in range(8)]
        for kt in range(8):
            nc.sync.dma_start(kf[kt][:], kag[kt * 128:(kt + 1) * 128, :])
        v65 = {}
        for rb in range(4):
            for st in range(8):
                vt = kvf.tile([128, 260], bff, tag=f"v65_{rb}_{st}",
                              name=f"v65_{rb}_{st}")
                nc.vector.memset(vt[:], 1.0)
                src = vag[rb * 1024 + st * 128: rb * 1024 + st * 128 + 128, :]
                dst = vt[:].rearrange("p (w c) -> p w c", c=65)[:, :, 0:64]
                nc.sync.dma_start(dst, src.rearrange("p (w c) -> p w c", c=64))
                v65[(rb, st)] = vt
        qw = [attwp.tile([128, 1024], bff, tag=f"qw{kd}", name=f"qw{kd}")
              for kd in range(8)]
        ow = [attwp.tile([128, 1024], bff, tag=f"ow{hp}", name=f"ow{hp}")
              for hp in range(8)]
        qbt = attwp.tile([1, 1024], bff, tag="qbt", name="qbt")
        obt = attwp.tile([1, 1024], bff, tag="obt", name="obt")
        nc.sync.dma_start(qbt[:], D['qb'][:])
        nc.sync.dma_start(obt[:], D['ob'][:])
        for kd in range(8):
            nc.sync.dma_start(qw[kd][:], D['qwT'][kd * 128:(kd + 1) * 128, :])
            nc.sync.dma_start(ow[kd][:], D['owT'][kd * 128:(kd + 1) * 128, :])
        qt = [attwp.tile([128, 256], bff, tag=f"q{mt}", name=f"q{mt}")
              for mt in range(8)]
        with tc.tile_pool(name="qpsp", bufs=3, space="PSUM") as qpsp:
            for mt in range(8):
                ps = qpsp.tile([128, 256], f32, tag="qps", name="qps")
                for kd in range(8):
                    mm(ps[:], qw[kd][:, mt * 128:(mt + 1) * 128], h1T[kd][:],
                       start=(kd == 0), stop=False)
                mm(ps[:], qbt[0:1, mt * 128:(mt + 1) * 128], ones_b[0:1, 0:256],
                   start=False, stop=True)
                nc.scalar.activation(qt[mt][:], ps[:], AF.Copy)
        o_bf = [attwp.tile([128, 256], bff, tag=f"obf{hp}", name=f"obf{hp}")
                for hp in range(8)]
        hl_cm = tc.tile_pool(name="hlps", bufs=3, space="PSUM")
        aps = hl_cm.__enter__()
        avps_cm = tc.tile_pool(name="avps", bufs=2, space="PSUM")
        avps = avps_cm.__enter__()
        bcps_cm = tc.tile_pool(name="bcps", bufs=2, space="PSUM")
        bcps = bcps_cm.__enter__()
        for hh in range(16):
            kt, koff = hh // 2, (hh % 2) * 64
            rb, w = hh // 4, hh % 4
            av = avps.tile([65, 256], f32, tag="av", name="av")
            for st in range(8):
                ps = aps.tile([128, 256], f32, tag="sc", name="scp")
                mm(ps[:], kf[kt][koff:koff + 64, st * 128:(st + 1) * 128],
                   qt[kt][koff:koff + 64, :], start=True, stop=True)
                E = asb.tile([128, 256], bff, tag="E", name="E")
                nc.scalar.activation(E[:], ps[:], AF.Exp, bias=maskb[:, st:st + 1])
                mm(av[:], v65[(rb, st)][:, w * 65:(w + 1) * 65], E[:],
                   start=(st == 0), stop=(st == 7))
            rden = asb.tile([1, 256], f32, tag="rden", name="rden")
            nc.vector.reciprocal(rden[:], av[64:65, :])
            bc = bcps.tile([64, 256], f32, tag="bc", name="bc")
            mm(bc[:], ones_f[0:1, 0:64], rden[:], start=True, stop=True)
            bcs = asb.tile([64, 256], f32, tag="bcs", name="bcs")
            nc.scalar.activation(bcs[:], bc[:], AF.Copy)
            nc.vector.tensor_tensor(
                o_bf[hh // 2][(hh % 2) * 64:(hh % 2) * 64 + 64, :],
                av[0:64, :], bcs[:], OP.mult)
        bcps_cm.__exit__(None, None, None)
        avps_cm.__exit__(None, None, None)
        hl_cm.__exit__(None, None, None)
        op_cm = tc.tile_pool(name="opps2", bufs=2, space="PSUM")
        opps2 = op_cm.__enter__()
        for t in range(2):
            r2 = h2p.tile([128, 1024], f32, tag="r2", name="r2")
            for nh in range(2):
                ps = opps2.tile([128, 512], f32, tag="ops2", name="ops2")
                for hp in range(8):
                    mm(ps[:], o_bf[hp][:, t * 128:(t + 1) * 128],
                       ow[hp][:, _ns(nh)], start=(hp == 0), stop=False)
                mm(ps[:], ones_b[0:1, 0:128], obt[0:1, _ns(nh)],
                   start=False, stop=True)
                nc.vector.tensor_tensor(r2[:, _ns(nh)], h1[t][:, _ns(nh)],
                                        ps[:], OP.add)
            rmsnorm(h2[t][:], r2[:], h2p, D['nw2row'], flags['has_nw2'])
        op_cm.__exit__(None, None, None)
    app_cm.__exit__(None, None, None)

    # ============== Phase FFN ==============
    s_w1, s_w2 = flags['s_w1'], flags['s_w2']

    def act_quant(pool, src_tiles, sc_imm):
        ams = []
        for j, srct in enumerate(src_tiles):
            am = pool.tile([128, 1], f32, tag="qam", name="qam")
            nc.vector.tensor_reduce(am[:], srct[:], mybir.AxisListType.X, OP.max,
                                    apply_absolute_value=True)
            ams.append(am)
        am = ams[0]
        for other in ams[1:]:
            nc.vector.tensor_tensor(am[:], am[:], other[:], OP.max)
        s = pool.tile([128, 1], f32, tag="qs", name="qs")
        nc.vector.tensor_scalar(s[:], am[:], 1e-5, 1.0 / 127.0, OP.max, OP.mult)
        sc = pool.tile([128, 1], f32, tag="qsc", name="qsc")
        nc.vector.tensor_scalar(sc[:], s[:], sc_imm, None, OP.mult)
        sr = pool.tile([128, 1], f32, tag="qsr", name="qsr")
        nc.vector.reciprocal(sr[:], s[:])
        q_tiles = []
        for srct in src_tiles:
            w = srct.shape[1]
            d = pool.tile([128, w], f32, tag="qd", name="qd")
            nc.vector.tensor_scalar(d[:], srct[:], sr[:, 0:1], None, OP.mult)
            sg = pool.tile([128, w], f32, tag="qsg", name="qsg", bufs=1)
            nc.scalar.activation(sg[:], d[:], AF.Sign)
            nc.vector.scalar_tensor_tensor(d[:], sg[:], 0.5, d[:], OP.mult, OP.add)
            nc.vector.tensor_scalar(d[:], d[:], 127.49, -127.49, OP.min, OP.max)
            qi = pool.tile([128, w], i32, tag="qi", name="qi", bufs=1)
            nc.vector.tensor_copy(qi[:], d[:])
            qf = pool.tile([128, w], f32, tag="qf", name="qf", bufs=4)
            nc.vector.tensor_copy(qf[:], qi[:])
            q_tiles.append(qf)
        return q_tiles, sc

    r3 = [ptile([128, 1024], f32, f"r3_{t}") for t in range(2)]
    with tc.tile_pool(name="fq", bufs=2) as fq, \
         tc.tile_pool(name="ftr", bufs=1) as ftr, \
         tc.tile_pool(name="fw", bufs=3) as fw, \
         tc.tile_pool(name="fmid", bufs=1) as fmid:
        xq, sc1 = [], []
        for t in range(2):
            qts, sc = act_quant(fq, [h2[t]], s_w1)
            xq.append(qts[0]); sc1.append(sc)
        xqT = [ftr.tile([128, 256], bff, tag=f"xqT{d}", name=f"xqT{d}")
               for d in range(8)]
        with tc.tile_pool(name="ftps", bufs=2, space="PSUM") as ftps:
            for d in range(8):
                ps = ftps.tile([128, 256], f32, tag="ftr", name="ftrp")
                for t in range(2):
                    nc.tensor.transpose(ps[:, t * 128:(t + 1) * 128],
                                        xq[t][:, d * 128:(d + 1) * 128], ident[:])
                nc.scalar.activation(xqT[d][:], ps[:], AF.Copy)
        hmid = [[fmid.tile([128, 2048], f32, tag=f"hmid{t}_{fh}",
                           name=f"hmid{t}_{fh}") for fh in range(2)]
                for t in range(2)]
        with tc.tile_pool(name="f1ps", bufs=2, space="PSUM") as f1ps:
            for fh in range(2):
                pboth = [f1ps.tile([128, 2048], f32, tag="f1", name=f"f1p{t}")
                         for t in range(2)]
                for kd in range(8):
                    w1t = fw.tile([128, 2048], bff, tag="w1t", name=f"w1t{kd}",
                                  bufs=3)
                    nc.sync.dma_start(
                        w1t[:], D['w1qT'][kd * 128:(kd + 1) * 128,
                                          fh * 2048:(fh + 1) * 2048])
                    for t in range(2):
                        for nsb in range(4):
                            mm(pboth[t][:, nsb * 512:(nsb + 1) * 512],
                               xqT[kd][:, t * 128:(t + 1) * 128],
                               w1t[:, nsb * 512:(nsb + 1) * 512],
                               start=(kd == 0), stop=(kd == 7))
                for t in range(2):
                    ps = pboth[t]
                    if sim_funcs or flags['has_b1']:
                        tmp = fmid.tile([128, 2048], f32, tag="b1tmp",
                                        name="b1tmp")
                        nc.vector.tensor_scalar(tmp[:], ps[:], sc1[t][:, 0:1],
                                                None, OP.mult)
                        if flags['has_b1']:
                            b1r = fmid.tile([1, 2048], f32, tag="b1r", name="b1r")
                            nc.sync.dma_start(
                                b1r[:], D['b1row'][0:1, fh * 2048:(fh + 1) * 2048])
                            b1ps = f1ps.tile([128, 2048], f32, tag="f1",
                                             name="b1ps")
                            for nsb in range(4):
                                mm(b1ps[:, nsb * 512:(nsb + 1) * 512],
                                   ones_f[0:1, 0:128],
                                   b1r[0:1, nsb * 512:(nsb + 1) * 512],
                                   start=True, stop=True)
                            nc.vector.tensor_tensor(tmp[:], tmp[:], b1ps[:],
                                                    OP.add)
                        if sim_funcs:
                            _gelu_sim(nc, mybir, fmid, hmid[t][fh][:], tmp)
                        else:
                            nc.scalar.activation(hmid[t][fh][:], tmp[:],
                                                 AF.Gelu_apprx_tanh)
                    else:
                        nc.scalar.activation(hmid[t][fh][:], ps[:],
                                             AF.Gelu_apprx_tanh,
                                             scale=sc1[t][:, 0:1])
        q2, sc2 = [], []
        for t in range(2):
            qts, sc = act_quant(fq, hmid[t], s_w2)
            q2.append(qts); sc2.append(sc)
        q2T = [ftr.tile([128, 256], bff, tag=f"q2T{fd}", name=f"q2T{fd}")
               for fd in range(32)]
        with tc.tile_pool(name="ftps2", bufs=2, space="PSUM") as ftps2:
            for fd in range(32):
                fh, j = fd // 16, fd % 16
                ps = ftps2.tile([128, 256], f32, tag="ftr2", name="ftr2p")
                for t in range(2):
                    nc.tensor.transpose(ps[:, t * 128:(t + 1) * 128],
                                        q2[t][fh][:, j * 128:(j + 1) * 128],
                                        ident[:])
                nc.scalar.activation(q2T[fd][:], ps[:], AF.Copy)
        with tc.tile_pool(name="f2ps", bufs=4, space="PSUM") as f2ps:
            pss = {}
            for t in range(2):
                for nsb in range(2):
                    pss[(t, nsb)] = f2ps.tile([128, 512], f32, tag="f2",
                                              name=f"f2_{t}_{nsb}")
            for kfi in range(32):
                w2t = fw.tile([128, 1024], bff, tag="w2t", name="w2t", bufs=3)
                nc.sync.dma_start(w2t[:], D['w2qT'][kfi * 128:(kfi + 1) * 128, :])
                for t in range(2):
                    for nsb in range(2):
                        mm(pss[(t, nsb)][:], q2T[kfi][:, t * 128:(t + 1) * 128],
                           w2t[:, _ns(nsb)], start=(kfi == 0), stop=(kfi == 31))
            for t in range(2):
                for nsb in range(2):
                    nc.vector.scalar_tensor_tensor(
                        r3[t][:, _ns(nsb)], pss[(t, nsb)][:], sc2[t][:, 0:1],
                        h2[t][:, _ns(nsb)], OP.mult, OP.add)
                if flags['has_b2']:
                    b2r = fmid.tile([1, 1024], f32, tag="b2r", name="b2r")
                    nc.sync.dma_start(b2r[:], D['b2row'][:])
                    b2ps = f2ps.tile([128, 512], f32, tag="f2", name="b2ps")
                    for nsb in range(2):
                        mm(b2ps[:], ones_f[0:1, 0:128],
                           b2r[0:1, _ns(nsb)], start=True, stop=True)
                        nc.vector.tensor_tensor(r3[t][:, _ns(nsb)],
                                                r3[t][:, _ns(nsb)], b2ps[:],
                                                OP.add)

    # ============== Phase NORM3 + output ==============
    with tc.tile_pool(name="n3", bufs=2) as n3p:
        for t in range(2):
            o3 = n3p.tile([128, 1024], f32, tag="o3", name="o3")
            rmsnorm(o3[:], r3[t][:], n3p, D['nw3row'], flags['has_nw3'])
            nc.sync.dma_start(D['out_tok'][t * 128:(t + 1) * 128, :], o3[:])

    es.close()


def _gelu_sim(nc, mybir, pool, dst, x):
    AF = mybir.ActivationFunctionType
    OP = mybir.AluOpType
    f32 = mybir.dt.float32
    w = x.shape[1]
    x3 = pool.tile([128, w], f32, tag="gx3", name="gx3")
    nc.scalar.activation(x3[:], x[:], AF.Square)
    nc.vector.tensor_tensor(x3[:], x3[:], x[:], OP.mult)
    targ = pool.tile([128, w], f32, tag="gtg", name="gtg")
    nc.vector.scalar_tensor_tensor(targ[:], x3[:], 0.044715, x[:], OP.mult, OP.add)
    sg = pool.tile([128, w], f32, tag="gsg", name="gsg")
    c2 = float(2.0 * np.sqrt(2.0 / np.pi))
    nc.scalar.activation(sg[:], targ[:], AF.Sigmoid, scale=c2)
    nc.vector.tensor_tensor(dst, sg[:], x[:], OP.mult)


def kernel(_trace=False, _sim_funcs=False, **inputs) -> np.ndarray:
    from concourse import bass_utils

    in_maps, flags = _host_prep(inputs)
    key = (tuple(sorted(flags.items())), _sim_funcs)
    if key not in _CACHE:
        _CACHE[key] = _build(flags, sim_funcs=_sim_funcs)
    nc = _CACHE[key]

    res = bass_utils.run_bass_kernel_spmd(
        nc, in_maps, core_ids=list(range(N_CORES)), trace=_trace)
    out = np.zeros((B, TGT, D_MODEL), np.float32)
    for c in range(N_CORES):
        b, r = c // N_TP, c % N_TP
        out[b, r * TOK:(r + 1) * TOK, :] = res.results[c]["out_tok"]
    kernel._last_exec_time_ns = res.exec_time_ns
    return out



# revision 39
# speedup vs baseline: 1.7336x; 1.7336x over previous
"""Self-contained Trainium2 Bass kernel for nn_DecoderLayer_30855045055049.

Sharding: 2 DP groups over batch (cores 0-3 -> b=0, cores 4-7 -> b=1), 4-way
TP within each group. Mamba d_inner-sharded (512 ch/core; selective scan via
GPSIMD tensor_tensor_scan per (ch-tile, state)); xproj partial -> AllReduce;
out_proj partial stored [t, dm] -> ReduceScatter over tokens (256 tok/core).
Attention: k/v head-sharded + AllGather, then token-parallel over own 256
tokens (softmax without max-subtraction; bf16 compute; fused av+denominator
via a ones-column appended to v). FFN token-parallel with exact-integer
int8-activation x ternary-weight bf16 matmuls. Final rmsnorm token-parallel;
host assembles the token shards.
"""
import numpy as np
import ml_dtypes

B, TGT, SRC = 2, 1024, 1024
D_MODEL, D_INNER, D_STATE, D_CONV, DT_RANK, D_FF, N_HEADS = 1024, 2048, 16, 4, 64, 4096, 16
EPS = 1e-6
N_CORES, N_TP = 8, 4
CH = D_INNER // N_TP          # 512 channels/core
TOK = TGT // N_TP             # 256 tokens/core
HD = D_MODEL // N_HEADS       # 64
RG = [[0, 1, 2, 3], [4, 5, 6, 7]]

_CACHE = {}


def _bf16(x):
    return np.asarray(x, np.float32).astype(ml_dtypes.bfloat16)


def _ns(nh):
    return slice(nh * 512, (nh + 1) * 512)


def _host_prep(inputs):
    f = lambda k: np.ascontiguousarray(np.asarray(inputs[k], np.float32))
    x = f('x'); enc = f('encoder_out')
    mask = np.asarray(inputs['encoder_mask'])
    in_w = f('mamba_in_w'); conv_w = f('mamba_conv_w'); conv_b = f('mamba_conv_b')
    xproj_w = f('mamba_xproj_w'); dt_w = f('mamba_dt_w'); dt_b = f('mamba_dt_b')
    A = -np.exp(f('mamba_A_log'))
    Dp = f('mamba_D'); out_w = f('mamba_out_w')
    q_w = f('q_w'); q_b = f('q_b'); k_w = f('k_w'); k_b = f('k_b')
    v_w = f('v_w'); v_b = f('v_b'); o_w = f('o_w'); o_b = f('o_b')
    w1 = f('ffn_w1'); b1 = f('ffn_b1'); w2 = f('ffn_w2'); b2 = f('ffn_b2')
    nw1 = f('norm1_w'); nw2 = f('norm2_w'); nw3 = f('norm3_w')

    def wquant(w):
        s = max(float(np.mean(np.abs(w))), 1e-5)
        return np.clip(np.round(w / s), -1.0, 1.0), np.float32(s)

    w1q, s_w1 = wquant(w1)
    w2q, s_w2 = wquant(w2)
    mask_bias = np.where(mask, 0.0, -1e9).astype(np.float32)

    flags = dict(
        has_b1=bool(np.any(b1 != 0)), has_b2=bool(np.any(b2 != 0)),
        has_nw1=bool(np.any(nw1 != 1)), has_nw2=bool(np.any(nw2 != 1)),
        has_nw3=bool(np.any(nw3 != 1)),
        s_w1=float(s_w1), s_w2=float(s_w2),
    )

    ident = np.eye(128, dtype=np.float32)
    hscale = 1.0 / np.sqrt(HD)

    in_maps = []
    for c in range(N_CORES):
        b, r = c // N_TP, c % N_TP
        chs = slice(r * CH, (r + 1) * CH)
        toks = slice(r * TOK, (r + 1) * TOK)
        hsl = slice(r * (N_HEADS // N_TP) * HD, (r + 1) * (N_HEADS // N_TP) * HD)

        convdiag = np.zeros((16, 128, 128), np.float32)
        Ddiag = np.zeros((4, 128, 128), np.float32)
        for i in range(4):
            cw = conv_w[r * CH + i * 128: r * CH + (i + 1) * 128, 0, :]
            for k in range(D_CONV):
                np.fill_diagonal(convdiag[i * 4 + k], cw[:, k])
            np.fill_diagonal(Ddiag[i], Dp[chs][i * 128:(i + 1) * 128])
        A_cols = np.empty((128, 64), np.float32)
        for i in range(4):
            A_cols[:, i * 16:(i + 1) * 16] = A[chs][i * 128:(i + 1) * 128, :]

        m = dict(
            xT=x[b].T, x_tok=x[b, toks],
            encT_bf=_bf16(enc[b].T),
            inw_uT=in_w[chs, :].T,
            inw_resT=in_w[D_INNER:, :][chs, :].T,
            convdiag=convdiag, Ddiag=Ddiag,
            cvb=conv_b[chs].reshape(4, 128).T,
            dtb=dt_b[chs].reshape(4, 128).T,
            A_cols=A_cols,
            xprojT=xproj_w[:, chs].T,
            dtwT=dt_w[chs, :].T,
            outwT=out_w[:, chs].T,
            qwT=_bf16(q_w.T * hscale), qb=_bf16((q_b * hscale).reshape(1, -1)),
            kwT=_bf16(k_w[hsl, :].T), kb=_bf16(k_b[hsl].reshape(1, -1)),
            vwT=_bf16(v_w[hsl, :].T), vb=_bf16(v_b[hsl].reshape(1, -1)),
            owT=_bf16(o_w.T), ob=_bf16(o_b.reshape(1, -1)),
            maskb=mask_bias[b].reshape(8, 128).T,
            w1qT=_bf16(w1q.T), w2qT=_bf16(w2q.T),
            b1row=b1.reshape(1, -1), b2row=b2.reshape(1, -1),
            nw1row=nw1.reshape(1, -1), nw2row=nw2.reshape(1, -1),
            nw3row=nw3.reshape(1, -1),
            ident=ident, ident_r=ident,
            ones_f=np.ones((1, 512), np.float32),
            ones_b=_bf16(np.ones((1, 512))),
        )
        in_maps.append({k: np.ascontiguousarray(v) for k, v in m.items()})
    return in_maps, flags


def _build(flags, sim_funcs=False, reps=1):
    import concourse.bacc as bacc
    import concourse.tile as tile
    from concourse import mybir

    dt = mybir.dt
    f32, bff, f32r = dt.float32, dt.bfloat16, dt.float32r

    nc = bacc.Bacc("TRN2", target_bir_lowering=False, debug=False,
                   num_devices=N_CORES)

    def din(name, shape, d=f32):
        return nc.dram_tensor(name, shape, d, kind="ExternalInput").ap()

    D = dict(
        xT=din("xT", [1024, 1024], f32r), x_tok=din("x_tok", [256, 1024]),
        encT_bf=din("encT_bf", [1024, 1024], bff),
        inw_uT=din("inw_uT", [1024, 512], f32r),
        inw_resT=din("inw_resT", [1024, 512], f32r),
        convdiag=din("convdiag", [16, 128, 128], f32r),
        Ddiag=din("Ddiag", [4, 128, 128], f32r),
        cvb=din("cvb", [128, 4]), dtb=din("dtb", [128, 4]),
        A_cols=din("A_cols", [128, 64]),
        xprojT=din("xprojT", [512, 96], f32r), dtwT=din("dtwT", [64, 512], f32r),
        outwT=din("outwT", [512, 1024], f32r),
        qwT=din("qwT", [1024, 1024], bff), qb=din("qb", [1, 1024], bff),
        kwT=din("kwT", [1024, 256], bff), kb=din("kb", [1, 256], bff),
        vwT=din("vwT", [1024, 256], bff), vb=din("vb", [1, 256], bff),
        owT=din("owT", [1024, 1024], bff), ob=din("ob", [1, 1024], bff),
        maskb=din("maskb", [128, 8]),
        w1qT=din("w1qT", [1024, 4096], bff), w2qT=din("w2qT", [4096, 1024], bff),
        b1row=din("b1row", [1, 4096]), b2row=din("b2row", [1, 1024]),
        nw1row=din("nw1row", [1, 1024]), nw2row=din("nw2row", [1, 1024]),
        nw3row=din("nw3row", [1, 1024]),
        ident=din("ident", [128, 128]), ident_r=din("ident_r", [128, 128], f32r),
        ones_f=din("ones_f", [1, 512]),
        ones_b=din("ones_b", [1, 512], bff),
        out_tok=nc.dram_tensor("out_tok", [256, 1024], f32,
                               kind="ExternalOutput").ap(),
    )

    with tile.TileContext(nc) as tc:
        for _ in range(reps):
            _emit(nc, tc, mybir, D, flags, sim_funcs)
    nc.compile()
    return nc


def _emit(nc, tc, mybir, D, flags, sim_funcs):
    from contextlib import ExitStack
    dt = mybir.dt
    f32, f32r, bff, i32 = dt.float32, dt.float32r, dt.bfloat16, dt.int32
    AF = mybir.ActivationFunctionType
    OP = mybir.AluOpType
    r32 = lambda ap: ap.bitcast(f32r)
    mm = nc.tensor.matmul

    es = ExitStack()
    const = es.enter_context(tc.tile_pool(name="const", bufs=1))
    persist = es.enter_context(tc.tile_pool(name="persist", bufs=1))
    dram = es.enter_context(tc.tile_pool(name="dram", bufs=1, space="DRAM"))
    mp_cm = tc.tile_pool(name="mamba_p", bufs=1)
    mp = mp_cm.__enter__()
    app = None

    def ctile(shape, d, name):
        t = const.tile(shape, d, tag=name, name=name)
        return t

    def ptile(shape, d, name):
        return persist.tile(shape, d, tag=name, name=name)

    def mtile(shape, d, name):
        return mp.tile(shape, d, tag=name, name=name)

    def atile(shape, d, name):
        return app.tile(shape, d, tag=name, name=name)
    app_cm = None

    # constants
    ident = ctile([128, 128], f32, "ident"); nc.sync.dma_start(ident[:], D['ident'][:])
    ident_r = ctile([128, 128], f32r, "ident_r")
    nc.sync.dma_start(ident_r[:], D['ident_r'][:])
    ones_f = ctile([1, 512], f32, "ones_f"); nc.sync.dma_start(ones_f[:], D['ones_f'][:])
    ones_b = ctile([1, 512], bff, "ones_b"); nc.sync.dma_start(ones_b[:], D['ones_b'][:])
    acol = ctile([128, 64], f32, "acol"); nc.sync.dma_start(acol[:], D['A_cols'][:])
    dtbt = ctile([128, 4], f32, "dtbt"); nc.sync.dma_start(dtbt[:], D['dtb'][:])
    cvbt = ctile([128, 4], f32, "cvbt"); nc.sync.dma_start(cvbt[:], D['cvb'][:])
    maskb = ctile([128, 8], f32, "maskb"); nc.sync.dma_start(maskb[:], D['maskb'][:])

    def silu(dst, src, pool, bias=None):
        if sim_funcs:
            shifted = pool.tile([dst.shape[0], dst.shape[1]], f32, tag="silu_t",
                                name="silu_t", bufs=1)
            if bias is None:
                nc.vector.tensor_copy(shifted[:], src)
            else:
                nc.vector.tensor_scalar(shifted[:], src, bias, None, OP.add)
            sg = pool.tile([dst.shape[0], dst.shape[1]], f32, tag="silu_s",
                           name="silu_s", bufs=1)
            nc.scalar.activation(sg[:], shifted[:], AF.Sigmoid)
            nc.vector.tensor_tensor(dst, sg[:], shifted[:], OP.mult)
        else:
            nc.scalar.activation(dst, src, AF.Silu,
                                 bias=0.0 if bias is None else bias)

    # DRAM bounce buffers
    proj_in = dram.tile([96, 1024], f32, tag="proj_in", name="proj_in")
    proj_out = dram.tile([96, 1024], f32, tag="proj_out", name="proj_out")
    mpart = dram.tile([1024, 1024], f32, tag="mpart", name="mpart")
    mout = dram.tile([256, 1024], f32, tag="mout", name="mout")
    ksh = dram.tile([256, 1024], bff, tag="ksh", name="ksh")
    kag = dram.tile([1024, 1024], bff, tag="kag", name="kag")
    vsh = dram.tile([1024, 256], bff, tag="vsh", name="vsh")
    vag = dram.tile([4096, 256], bff, tag="vag", name="vag")

    # ============== Phase KV (first, so AllGathers launch early) ==============
    with tc.tile_pool(name="enc", bufs=1) as encp, \
         tc.tile_pool(name="kvw", bufs=1) as kvwp, \
         tc.tile_pool(name="kvps", bufs=2, space="PSUM") as kvps, \
         tc.tile_pool(name="kvsb", bufs=3) as kvsb:
        enct = [encp.tile([128, 1024], bff, tag=f"enc{kd}", name=f"enc{kd}")
                for kd in range(8)]
        kwt = [kvwp.tile([128, 256], bff, tag=f"kw{kd}", name=f"kw{kd}")
               for kd in range(8)]
        vwt = [kvwp.tile([128, 256], bff, tag=f"vw{kd}", name=f"vw{kd}")
               for kd in range(8)]
        kbt = kvwp.tile([1, 256], bff, tag="kbt", name="kbt")
        vbt = kvwp.tile([1, 256], bff, tag="vbt", name="vbt")
        nc.sync.dma_start(kbt[:], D['kb'][:])
        nc.sync.dma_start(vbt[:], D['vb'][:])
        for kd in range(8):
            nc.sync.dma_start(enct[kd][:], D['encT_bf'][kd * 128:(kd + 1) * 128, :])
            nc.sync.dma_start(kwt[kd][:], D['kwT'][kd * 128:(kd + 1) * 128, :])
            nc.sync.dma_start(vwt[kd][:], D['vwT'][kd * 128:(kd + 1) * 128, :])
        for mt in range(2):
            for nh in range(2):
                ps = kvps.tile([128, 512], f32, tag="kv", name="kvp")
                for kd in range(8):
                    mm(ps[:], kwt[kd][:, mt * 128:(mt + 1) * 128],
                       enct[kd][:, _ns(nh)], start=(kd == 0), stop=False)
                mm(ps[:], kbt[0:1, mt * 128:(mt + 1) * 128], ones_b[0:1, 0:512],
                   start=False, stop=True)
                sb = kvsb.tile([128, 512], bff, tag="kvsb", name="kvsbt")
                nc.scalar.activation(sb[:], ps[:], AF.Copy)
                nc.sync.dma_start(
                    ksh[mt * 128:(mt + 1) * 128, _ns(nh)], sb[:])
        for st in range(8):
            ps = kvps.tile([128, 256], f32, tag="kv", name="kvp2")
            for kd in range(8):
                mm(ps[:], enct[kd][:, st * 128:(st + 1) * 128], vwt[kd][:],
                   start=(kd == 0), stop=False)
            mm(ps[:], ones_b[0:1, 0:128], vbt[:], start=False, stop=True)
            sb = kvsb.tile([128, 256], bff, tag="kvsb", name="kvsbt2")
            nc.scalar.activation(sb[:], ps[:], AF.Copy)
            nc.sync.dma_start(vsh[st * 128:(st + 1) * 128, :], sb[:])
        nc.gpsimd.collective_compute("AllGather", OP.bypass, replica_groups=RG,
                                     ins=[ksh.opt()], outs=[kag.opt()])
        nc.gpsimd.collective_compute("AllGather", OP.bypass, replica_groups=RG,
                                     ins=[vsh.opt()], outs=[vag.opt()])

    # ============== Phase M1: in_proj, conv+silu, xproj + AllReduce ==========
    u_act = [mp.tile([128, 1024], f32r, tag=f"uact{i}", name=f"uact{i}")
             for i in range(4)]
    res_s = [mtile([128, 1024], f32, f"res{i}") for i in range(4)]
    proj_f = mtile([64, 1024], f32, "proj_f")
    proj_r = mtile([64, 1024], f32r, "proj_r")

    with tc.tile_pool(name="xt", bufs=1) as xtp, \
         tc.tile_pool(name="inw", bufs=18) as inwp, \
         tc.tile_pool(name="m1ps", bufs=2, space="PSUM") as m1ps, \
         tc.tile_pool(name="upad", bufs=1) as upadp, \
         tc.tile_pool(name="cdg", bufs=2) as cdgp, \
         tc.tile_pool(name="silup", bufs=2) as silup:
        xt = [xtp.tile([128, 1024], f32r, tag=f"xt{kd}", name=f"xt{kd}")
              for kd in range(8)]
        for kd in range(8):
            nc.sync.dma_start(xt[kd][:], D['xT'][kd * 128:(kd + 1) * 128, :])
        upads = [upadp.tile([128, 1028], f32r, tag=f"upad{i}", name=f"upad{i}")
                 for i in range(4)]
        for i in range(4):
            nc.vector.memset(upads[i][:, 0:3].bitcast(f32), 0.0)
        for i in range(4):
            iwu = [inwp.tile([128, 128], f32r, tag="iw", name=f"iwu{i}_{kd}")
                   for kd in range(8)]
            for kd in range(8):
                nc.sync.dma_start(iwu[kd][:],
                                  D['inw_uT'][kd * 128:(kd + 1) * 128,
                                              i * 128:(i + 1) * 128])
            for nh in range(2):
                ps = m1ps.tile([128, 512], f32, tag="m1", name="m1u")
                for kd in range(8):
                    mm(ps[:], iwu[kd][:], xt[kd][:, _ns(nh)],
                       start=(kd == 0), stop=(kd == 7))
                nc.scalar.activation(
                    upads[i][:, 3 + nh * 512: 3 + nh * 512 + 512], ps[:], AF.Copy)
        for i in range(4):
            iwr = [inwp.tile([128, 128], f32r, tag="iw", name=f"iwr{i}_{kd}")
                   for kd in range(8)]
            for kd in range(8):
                nc.sync.dma_start(iwr[kd][:],
                                  D['inw_resT'][kd * 128:(kd + 1) * 128,
                                                i * 128:(i + 1) * 128])
            for nh in range(2):
                ps = m1ps.tile([128, 512], f32, tag="m1", name="m1r")
                for kd in range(8):
                    mm(ps[:], iwr[kd][:], xt[kd][:, _ns(nh)],
                       start=(kd == 0), stop=(kd == 7))
                nc.scalar.activation(res_s[i][:, _ns(nh)], ps[:], AF.Copy)
        for i in range(4):
            cdg = [cdgp.tile([128, 128], f32r, tag="cdg", name=f"cdg{i}_{k}",
                             bufs=5) for k in range(4)]
            for k in range(4):
                nc.sync.dma_start(cdg[k][:], D['convdiag'][i * 4 + k, :, :])
            for nh in range(2):
                ps = m1ps.tile([128, 512], f32, tag="m1", name="m1c")
                for k in range(4):
                    mm(ps[:], cdg[k][:],
                       upads[i][:, k + nh * 512: k + nh * 512 + 512],
                       start=(k == 0), stop=(k == 3))
                silu(u_act[i][:, _ns(nh)], ps[:], silup, bias=cvbt[:, i:i + 1])
        xpw = [cdgp.tile([128, 96], f32r, tag="xpw", name=f"xpw{i}", bufs=4)
               for i in range(4)]
        for i in range(4):
            nc.sync.dma_start(xpw[i][:], D['xprojT'][i * 128:(i + 1) * 128, :])
        for nh in range(2):
            ps = m1ps.tile([96, 512], f32, tag="m1", name="m1x")
            for i in range(4):
                mm(ps[:], xpw[i][:], u_act[i][:, _ns(nh)],
                   start=(i == 0), stop=(i == 3))
            sb = cdgp.tile([96, 512], f32, tag="xpsb", name="xpsb")
            nc.vector.tensor_copy(sb[:], ps[:])
            nc.sync.dma_start(proj_in[:, _ns(nh)], sb[:])
        nc.gpsimd.collective_compute("AllReduce", OP.add, replica_groups=RG,
                                     ins=[proj_in.opt()], outs=[proj_out.opt()])
        nc.sync.dma_start(proj_f[:], proj_out[0:64, :])
        nc.scalar.activation(proj_r[:], proj_f[:], AF.Copy)

    # ============== Phase M2: delta, du ==============
    delta = [mtile([128, 1024], f32, f"delta{i}") for i in range(4)]
    du = [mtile([128, 1024], f32, f"du{i}") for i in range(4)]
    with tc.tile_pool(name="m2ps", bufs=2, space="PSUM") as m2ps, \
         tc.tile_pool(name="m2sb", bufs=2) as m2sb:
        dtw = m2sb.tile([64, 512], f32r, tag="dtw", name="dtw", bufs=1)
        nc.sync.dma_start(dtw[:], D['dtwT'][:])
        for i in range(4):
            for nh in range(2):
                ps = m2ps.tile([128, 512], f32, tag="m2", name="m2d")
                mm(ps[:], dtw[:, i * 128:(i + 1) * 128],
                   proj_r[:, _ns(nh)], start=True, stop=True)
                tmp = m2sb.tile([128, 512], f32, tag="spe", name="spe")
                nc.scalar.activation(tmp[:], ps[:], AF.Exp, bias=dtbt[:, i:i + 1])
                nc.scalar.activation(delta[i][:, _ns(nh)], tmp[:], AF.Ln, bias=1.0)
            nc.vector.tensor_tensor(du[i][:], delta[i][:], u_act[i][:], OP.mult)

    # ============== Phase SCAN (s-outer; B/C via DMA broadcast) ==============
    y_gated = [mp.tile([128, 1024], f32r, tag=f"yg{i}", name=f"yg{i}")
               for i in range(4)]
    with tc.tile_pool(name="bcp", bufs=3) as bcp, \
         tc.tile_pool(name="yps", bufs=1, space="PSUM") as yps, \
         tc.tile_pool(name="ssb", bufs=2) as ssb, \
         tc.tile_pool(name="ddg", bufs=4) as ddgp:
        ys = [yps.tile([128, 1024], f32, tag=f"y{i}", name=f"y{i}")
              for i in range(4)]
        for i in range(4):
            dscale = ddgp.tile([128, 128], f32r, tag="ddg", name=f"ddg{i}")
            nc.sync.dma_start(dscale[:], D['Ddiag'][i, :, :])
            for nh in range(2):
                mm(ys[i][:, _ns(nh)], dscale[:], u_act[i][:, _ns(nh)],
                   start=True, stop=False)
        for s in range(16):
            Bt = bcp.tile([128, 1024], f32, tag="Bt", name="Bt")
            nc.sync.dma_start(Bt[:],
                              proj_out[64 + s:65 + s, :].to_broadcast((128, 1024)))
            Ct = bcp.tile([128, 1024], f32, tag="Ct", name="Ct")
            nc.sync.dma_start(Ct[:],
                              proj_out[80 + s:81 + s, :].to_broadcast((128, 1024)))
            for i in range(4):
                dA = ssb.tile([128, 1024], f32, tag="dA", name="dA")
                nc.scalar.activation(dA[:], delta[i][:], AF.Exp,
                                     scale=acol[:, i * 16 + s: i * 16 + s + 1])
                dBu = ssb.tile([128, 1024], f32, tag="dBu", name="dBu")
                nc.gpsimd.tensor_tensor(dBu[:], du[i][:], Bt[:], OP.mult)
                h = ssb.tile([128, 1024], f32, tag="h", name="h")
                nc.vector.tensor_tensor_scan(h[:], dA[:], dBu[:], 0.0,
                                             OP.mult, OP.add)
                hc = ssb.tile([128, 1024], f32r, tag="hc", name="hc")
                eng = nc.gpsimd if (s % 8 == 7) else nc.vector
                eng.tensor_tensor(hc[:], h[:], Ct[:], OP.mult)
                for nh in range(2):
                    mm(ys[i][:, _ns(nh)], ident_r[:], hc[:, _ns(nh)],
                       start=False, stop=(s == 15))
        for i in range(4):
            rs = ssb.tile([128, 1024], f32, tag="rsilu", name="rsilu", bufs=1)
            silu(rs[:], res_s[i][:], ssb)
            nc.vector.tensor_tensor(y_gated[i][:], ys[i][:], rs[:], OP.mult)

    # ============== Phase OUT_PROJ + ReduceScatter ==============
    with tc.tile_pool(name="ops", bufs=2, space="PSUM") as ops, \
         tc.tile_pool(name="osb", bufs=3) as osb, \
         tc.tile_pool(name="oww", bufs=1) as owwp:
        oww = [owwp.tile([128, 1024], f32r, tag=f"oww{i}", name=f"oww{i}")
               for i in range(4)]
        for i in range(4):
            nc.sync.dma_start(oww[i][:], D['outwT'][i * 128:(i + 1) * 128, :])
        for mt in range(8):
            for nh in range(2):
                ps = ops.tile([128, 512], f32, tag="op", name="opps")
                for i in range(4):
                    mm(ps[:], y_gated[i][:, mt * 128:(mt + 1) * 128],
                       oww[i][:, _ns(nh)], start=(i == 0), stop=(i == 3))
                sb = osb.tile([128, 512], f32, tag="osb", name="osbt")
                nc.vector.tensor_copy(sb[:], ps[:])
                nc.sync.dma_start(mpart[mt * 128:(mt + 1) * 128, _ns(nh)], sb[:])
        nc.gpsimd.collective_compute("ReduceScatter", OP.add, replica_groups=RG,
                                     ins=[mpart.opt()], outs=[mout.opt()])
    mp_cm.__exit__(None, None, None)

    # ============== rmsnorm helper ==============
    def rmsnorm(dst, r_ap, pool, nwrow_d, has_nw):
        sq = pool.tile([128, 1024], f32, tag="nsq", name="nsq")
        ssq = pool.tile([128, 1], f32, tag="nssq", name="nssq")
        nc.scalar.activation(sq[:], r_ap, AF.Square, accum_out=ssq[:])
        m = pool.tile([128, 1], f32, tag="nm", name="nm")
        nc.vector.tensor_scalar(m[:], ssq[:], 1.0 / 1024.0, EPS, OP.mult, OP.add)
        sr = pool.tile([128, 1], f32, tag="nsr", name="nsr")
        nc.scalar.activation(sr[:], m[:], AF.Sqrt)
        rinv = pool.tile([128, 1], f32, tag="nrinv", name="nrinv")
        nc.vector.reciprocal(rinv[:], sr[:])
        nc.vector.tensor_scalar(dst, r_ap, rinv[:, 0:1], None, OP.mult)
        if has_nw:
            nwr = pool.tile([1, 1024], f32, tag="nwr", name="nwr")
            nc.sync.dma_start(nwr[:], nwrow_d[:])
            nwb = pool.tile([128, 1024], f32, tag="nwb", name="nwb")
            nps = pool.tile([128, 1024], f32, tag="nwps", name="nwps",
                            space="PSUM")
            for nh in range(2):
                mm(nps[:, _ns(nh)], ones_f[0:1, 0:128],
                   nwr[0:1, _ns(nh)], start=True, stop=True)
            nc.vector.tensor_copy(nwb[:], nps[:])
            nc.vector.tensor_tensor(dst, dst, nwb[:], OP.mult)

    # ============== Phase H1 ==============
    app_cm = tc.tile_pool(name="attn_p", bufs=1)
    app = app_cm.__enter__()
    h1 = [atile([128, 1024], f32, f"h1_{t}") for t in range(2)]
    h1T = [atile([128, 256], bff, f"h1T{d}") for d in range(8)]
    with tc.tile_pool(name="h1p", bufs=2) as h1p, \
         tc.tile_pool(name="h1ps", bufs=2, space="PSUM") as h1ps:
        for t in range(2):
            mo = h1p.tile([128, 1024], f32, tag="mo", name="mo")
            nc.sync.dma_start(mo[:], mout[t * 128:(t + 1) * 128, :])
            xtk = h1p.tile([128, 1024], f32, tag="xtk", name="xtk")
            nc.sync.dma_start(xtk[:], D['x_tok'][t * 128:(t + 1) * 128, :])
            r1 = h1p.tile([128, 1024], f32, tag="r1", name="r1")
            nc.vector.tensor_tensor(r1[:], xtk[:], mo[:], OP.add)
            rmsnorm(h1[t][:], r1[:], h1p, D['nw1row'], flags['has_nw1'])
        for d in range(8):
            ps = h1ps.tile([128, 256], f32, tag="tr", name="trp")
            for t in range(2):
                nc.tensor.transpose(ps[:, t * 128:(t + 1) * 128],
                                    h1[t][:, d * 128:(d + 1) * 128], ident[:])
            nc.scalar.activation(h1T[d][:], ps[:], AF.Copy)

    # ============== Phase ATTN ==============
    h2 = [ptile([128, 1024], f32, f"h2_{t}") for t in range(2)]
    with tc.tile_pool(name="kvf", bufs=1) as kvf, \
         tc.tile_pool(name="attw", bufs=1) as attwp, \
         tc.tile_pool(name="asb", bufs=3) as asb, \
         tc.tile_pool(name="h2p", bufs=2) as h2p:
        kf = [kvf.tile([128, 1024], bff, tag=f"kf{kt}", name=f"kf{kt}")
              for kt in range(8)]
        for kt in range(8):
            nc.sync.dma_start(kf[kt][:], kag[kt * 128:(kt + 1) * 128, :])
        v65 = {}
        for rb in range(4):
            for st in range(8):
                vt = kvf.tile([128, 260], bff, tag=f"v65_{rb}_{st}",
                              name=f"v65_{rb}_{st}")
                nc.vector.memset(vt[:], 1.0)
                src = vag[rb * 1024 + st * 128: rb * 1024 + st * 128 + 128, :]
                dst = vt[:].rearrange("p (w c) -> p w c", c=65)[:, :, 0:64]
                nc.sync.dma_start(dst, src.rearrange("p (w c) -> p w c", c=64))
                v65[(rb, st)] = vt
        qw = [attwp.tile([128, 1024], bff, tag=f"qw{kd}", name=f"qw{kd}")
              for kd in range(8)]
        ow = [attwp.tile([128, 1024], bff, tag=f"ow{hp}", name=f"ow{hp}")
              for hp in range(8)]
        qbt = attwp.tile([1, 1024], bff, tag="qbt", name="qbt")
        obt = attwp.tile([1, 1024], bff, tag="obt", name="obt")
        nc.sync.dma_start(qbt[:], D['qb'][:])
        nc.sync.dma_start(obt[:], D['ob'][:])
        for kd in range(8):
            nc.sync.dma_start(qw[kd][:], D['qwT'][kd * 128:(kd + 1) * 128, :])
            nc.sync.dma_start(ow[kd][:], D['owT'][kd * 128:(kd + 1) * 128, :])
        qt = [attwp.tile([128, 256], bff, tag=f"q{mt}", name=f"q{mt}")
              for mt in range(8)]
        with tc.tile_pool(name="qpsp", bufs=3, space="PSUM") as qpsp:
            for mt in range(8):
                ps = qpsp.tile([128, 256], f32, tag="qps", name="qps")
                for kd in range(8):
                    mm(ps[:], qw[kd][:, mt * 128:(mt + 1) * 128], h1T[kd][:],
                       start=(kd == 0), stop=False)
                mm(ps[:], qbt[0:1, mt * 128:(mt + 1) * 128], ones_b[0:1, 0:256],
                   start=False, stop=True)
                nc.scalar.activation(qt[mt][:], ps[:], AF.Copy)
        o_bf = [attwp.tile([128, 256], bff, tag=f"obf{hp}", name=f"obf{hp}")
                for hp in range(8)]
        hl_cm = tc.tile_pool(name="hlps", bufs=3, space="PSUM")
        aps = hl_cm.__enter__()
        avps_cm = tc.tile_pool(name="avps", bufs=2, space="PSUM")
        avps = avps_cm.__enter__()
        bcps_cm = tc.tile_pool(name="bcps", bufs=2, space="PSUM")
        bcps = bcps_cm.__enter__()
        for hh in range(16):
            kt, koff = hh // 2, (hh % 2) * 64
            rb, w = hh // 4, hh % 4
            av = avps.tile([65, 256], f32, tag="av", name="av")
            for st in range(8):
                ps = aps.tile([128, 256], f32, tag="sc", name="scp")
                mm(ps[:], kf[kt][koff:koff + 64, st * 128:(st + 1) * 128],
                   qt[kt][koff:koff + 64, :], start=True, stop=True)
                E = asb.tile([128, 256], bff, tag="E", name="E")
                nc.scalar.activation(E[:], ps[:], AF.Exp, bias=maskb[:, st:st + 1])
                mm(av[:], v65[(rb, st)][:, w * 65:(w + 1) * 65], E[:],
                   start=(st == 0), stop=(st == 7))
            rden = asb.tile([1, 256], f32, tag="rden", name="rden")
            nc.vector.reciprocal(rden[:], av[64:65, :])
            bc = bcps.tile([64, 256], f32, tag="bc", name="bc")
            mm(bc[:], ones_f[0:1, 0:64], rden[:], start=True, stop=True)
            bcs = asb.tile([64, 256], f32, tag="bcs", name="bcs")
            nc.scalar.activation(bcs[:], bc[:], AF.Copy)
            nc.vector.tensor_tensor(
                o_bf[hh // 2][(hh % 2) * 64:(hh % 2) * 64 + 64, :],
                av[0:64, :], bcs[:], OP.mult)
        bcps_cm.__exit__(None, None, None)
        avps_cm.__exit__(None, None, None)
        hl_cm.__exit__(None, None, None)
        op_cm = tc.tile_pool(name="opps2", bufs=2, space="PSUM")
        opps2 = op_cm.__enter__()
        for t in range(2):
            r2 = h2p.tile([128, 1024], f32, tag="r2", name="r2")
            for nh in range(2):
                ps = opps2.tile([128, 512], f32, tag="ops2", name="ops2")
                for hp in range(8):
                    mm(ps[:], o_bf[hp][:, t * 128:(t + 1) * 128],
                       ow[hp][:, _ns(nh)], start=(hp == 0), stop=False)
                mm(ps[:], ones_b[0:1, 0:128], obt[0:1, _ns(nh)],
                   start=False, stop=True)
                nc.vector.tensor_tensor(r2[:, _ns(nh)], h1[t][:, _ns(nh)],
                                        ps[:], OP.add)
            rmsnorm(h2[t][:], r2[:], h2p, D['nw2row'], flags['has_nw2'])
        op_cm.__exit__(None, None, None)
    app_cm.__exit__(None, None, None)

    # ============== Phase FFN ==============
    s_w1, s_w2 = flags['s_w1'], flags['s_w2']

    def act_quant(pool, src_tiles, sc_imm):
        ams = []
        for j, srct in enumerate(src_tiles):
            am = pool.tile([128, 1], f32, tag="qam", name="qam")
            nc.vector.tensor_reduce(am[:], srct[:], mybir.AxisListType.X, OP.max,
                                    apply_absolute_value=True)
            ams.append(am)
        am = ams[0]
        for other in ams[1:]:
            nc.vector.tensor_tensor(am[:], am[:], other[:], OP.max)
        s = pool.tile([128, 1], f32, tag="qs", name="qs")
        nc.vector.tensor_scalar(s[:], am[:], 1e-5, 1.0 / 127.0, OP.max, OP.mult)
        sc = pool.tile([128, 1], f32, tag="qsc", name="qsc")
        nc.vector.tensor_scalar(sc[:], s[:], sc_imm, None, OP.mult)
        sr = pool.tile([128, 1], f32, tag="qsr", name="qsr")
        nc.vector.reciprocal(sr[:], s[:])
        q_tiles = []
        for srct in src_tiles:
            w = srct.shape[1]
            d = pool.tile([128, w], f32, tag="qd", name="qd")
            nc.vector.tensor_scalar(d[:], srct[:], sr[:, 0:1], None, OP.mult)
            sg = pool.tile([128, w], f32, tag="qsg", name="qsg", bufs=1)
            nc.scalar.activation(sg[:], d[:], AF.Sign)
            nc.vector.scalar_tensor_tensor(d[:], sg[:], 0.5, d[:], OP.mult, OP.add)
            nc.vector.tensor_scalar(d[:], d[:], 127.49, -127.49, OP.min, OP.max)
            qi = pool.tile([128, w], i32, tag="qi", name="qi", bufs=1)
            nc.vector.tensor_copy(qi[:], d[:])
            qf = pool.tile([128, w], f32, tag="qf", name="qf", bufs=4)
            nc.vector.tensor_copy(qf[:], qi[:])
            q_tiles.append(qf)
        return q_tiles, sc

    r3 = [ptile([128, 1024], f32, f"r3_{t}") for t in range(2)]
    with tc.tile_pool(name="fq", bufs=2) as fq, \
         tc.tile_pool(name="ftr", bufs=1) as ftr, \
         tc.tile_pool(name="fw", bufs=3) as fw, \
         tc.tile_pool(name="fmid", bufs=1) as fmid:
        xq, sc1 = [], []
        for t in range(2):
            qts, sc = act_quant(fq, [h2[t]], s_w1)
            xq.append(qts[0]); sc1.append(sc)
        xqT = [ftr.tile([128, 256], bff, tag=f"xqT{d}", name=f"xqT{d}")
               for d in range(8)]
        with tc.tile_pool(name="ftps", bufs=2, space="PSUM") as ftps:
            for d in range(8):
                ps = ftps.tile([128, 256], f32, tag="ftr", name="ftrp")
                for t in range(2):
                    nc.tensor.transpose(ps[:, t * 128:(t + 1) * 128],
                                        xq[t][:, d * 128:(d + 1) * 128], ident[:])
                nc.scalar.activation(xqT[d][:], ps[:], AF.Copy)
        hmid = [[fmid.tile([128, 2048], f32, tag=f"hmid{t}_{fh}",
                           name=f"hmid{t}_{fh}") for fh in range(2)]
                for t in range(2)]
        with tc.tile_pool(name="f1ps", bufs=2, space="PSUM") as f1ps:
            for fh in range(2):
                pboth = [f1ps.tile([128, 2048], f32, tag="f1", name=f"f1p{t}")
                         for t in range(2)]
                for kd in range(8):
                    w1t = fw.tile([128, 2048], bff, tag="w1t", name=f"w1t{kd}",
                                  bufs=3)
                    nc.sync.dma_start(
                        w1t[:], D['w1qT'][kd * 128:(kd + 1) * 128,
                                          fh * 2048:(fh + 1) * 2048])
                    for t in range(2):
                        for nsb in range(4):
                            mm(pboth[t][:, nsb * 512:(nsb + 1) * 512],
                               xqT[kd][:, t * 128:(t + 1) * 128],
                               w1t[:, nsb * 512:(nsb + 1) * 512],
                               start=(kd == 0), stop=(kd == 7))
                for t in range(2):
                    ps = pboth[t]
                    if sim_funcs or flags['has_b1']:
                        tmp = fmid.tile([128, 2048], f32, tag="b1tmp",
                                        name="b1tmp")
                        nc.vector.tensor_scalar(tmp[:], ps[:], sc1[t][:, 0:1],
                                                None, OP.mult)
                        if flags['has_b1']:
                            b1r = fmid.tile([1, 2048], f32, tag="b1r", name="b1r")
                            nc.sync.dma_start(
                                b1r[:], D['b1row'][0:1, fh * 2048:(fh + 1) * 2048])
                            b1ps = f1ps.tile([128, 2048], f32, tag="f1",
                                             name="b1ps")
                            for nsb in range(4):
                                mm(b1ps[:, nsb * 512:(nsb + 1) * 512],
                                   ones_f[0:1, 0:128],
                                   b1r[0:1, nsb * 512:(nsb + 1) * 512],
                                   start=True, stop=True)
                            nc.vector.tensor_tensor(tmp[:], tmp[:], b1ps[:],
                                                    OP.add)
                        if sim_funcs:
                            _gelu_sim(nc, mybir, fmid, hmid[t][fh][:], tmp)
                        else:
                            nc.scalar.activation(hmid[t][fh][:], tmp[:],
                                                 AF.Gelu_apprx_tanh)
                    else:
                        nc.scalar.activation(hmid[t][fh][:], ps[:],
                                             AF.Gelu_apprx_tanh,
                                             scale=sc1[t][:, 0:1])
        q2, sc2 = [], []
        for t in range(2):
            qts, sc = act_quant(fq, hmid[t], s_w2)
            q2.append(qts); sc2.append(sc)
        q2T = [ftr.tile([128, 256], bff, tag=f"q2T{fd}", name=f"q2T{fd}")
               for fd in range(32)]
        with tc.tile_pool(name="ftps2", bufs=2, space="PSUM") as ftps2:
            for fd in range(32):
                fh, j = fd // 16, fd % 16
                ps = ftps2.tile([128, 256], f32, tag="ftr2", name="ftr2p")
                for t in range(2):
                    nc.tensor.transpose(ps[:, t * 128:(t + 1) * 128],
                                        q2[t][fh][:, j * 128:(j + 1) * 128],
                                        ident[:])
                nc.scalar.activation(q2T[fd][:], ps[:], AF.Copy)
        with tc.tile_pool(name="f2ps", bufs=4, space="PSUM") as f2ps:
            pss = {}
            for t in range(2):
                for nsb in range(2):
                    pss[(t, nsb)] = f2ps.tile([128, 512], f32, tag="f2",
                                              name=f"f2_{t}_{nsb}")
            for kfi in range(32):
                w2t = fw.tile([128, 1024], bff, tag="w2t", name="w2t", bufs=3)
                nc.sync.dma_start(w2t[:], D['w2qT'][kfi * 128:(kfi + 1) * 128, :])
                for t in range(2):
                    for nsb in range(2):
                        mm(pss[(t, nsb)][:], q2T[kfi][:, t * 128:(t + 1) * 128],
                           w2t[:, _ns(nsb)], start=(kfi == 0), stop=(kfi == 31))
            for t in range(2):
                for nsb in range(2):
                    nc.vector.scalar_tensor_tensor(
                        r3[t][:, _ns(nsb)], pss[(t, nsb)][:], sc2[t][:, 0:1],
                        h2[t][:, _ns(nsb)], OP.mult, OP.add)
                if flags['has_b2']:
                    b2r = fmid.tile([1, 1024], f32, tag="b2r", name="b2r")
                    nc.sync.dma_start(b2r[:], D['b2row'][:])
                    b2ps = f2ps.tile([128, 512], f32, tag="f2", name="b2ps")
                    for nsb in range(2):
                        mm(b2ps[:], ones_f[0:1, 0:128],
                           b2r[0:1, _ns(nsb)], start=True, stop=True)
                        nc.vector.tensor_tensor(r3[t][:, _ns(nsb)],
                                                r3[t][:, _ns(nsb)], b2ps[:],
                                                OP.add)

    # ============== Phase NORM3 + output ==============
    with tc.tile_pool(name="n3", bufs=2) as n3p:
        for t in range(2):
            o3 = n3p.tile([128, 1024], f32, tag="o3", name="o3")
            rmsnorm(o3[:], r3[t][:], n3p, D['nw3row'], flags['has_nw3'])
            nc.sync.dma_start(D['out_tok'][t * 128:(t + 1) * 128, :], o3[:])

    es.close()


def _gelu_sim(nc, mybir, pool, dst, x):
    AF = mybir.ActivationFunctionType
    OP = mybir.AluOpType
    f32 = mybir.dt.float32
    w = x.shape[1]
    x3 = pool.tile([128, w], f32, tag="gx3", name="gx3")
    nc.scalar.activation(x3[:], x[:], AF.Square)
    nc.vector.tensor_tensor(x3[:], x3[:], x[:], OP.mult)
    targ = pool.tile([128, w], f32, tag="gtg", name="gtg")
    nc.vector.scalar_tensor_tensor(targ[:], x3[:], 0.044715, x[:], OP.mult, OP.add)
    sg = pool.tile([128, w], f32, tag="gsg", name="gsg")
    c2 = float(2.0 * np.sqrt(2.0 / np.pi))
    nc.scalar.activation(sg[:], targ[:], AF.Sigmoid, scale=c2)
    nc.vector.tensor_tensor(dst, sg[:], x[:], OP.mult)


def kernel(_trace=False, _sim_funcs=False, **inputs) -> np.ndarray:
    from concourse import bass_utils

    in_maps, flags = _host_prep(inputs)
    key = (tuple(sorted(flags.items())), _sim_funcs)
    if key not in _CACHE:
        _CACHE[key] = _build(flags, sim_funcs=_sim_funcs)
    nc = _CACHE[key]

    res = bass_utils.run_bass_kernel_spmd(
        nc, in_maps, core_ids=list(range(N_CORES)), trace=_trace)
    out = np.zeros((B, TGT, D_MODEL), np.float32)
    for c in range(N_CORES):
        b, r = c // N_TP, c % N_TP
        out[b, r * TOK:(r + 1) * TOK, :] = res.results[c]["out_tok"]
    kernel._last_exec_time_ns = res.exec_time_ns
    return out

